# revision 35
# baseline (speedup 1.0000x reference)
"""Single-token GQA decode attention (32 q heads / 8 kv heads, 8192-pos KV
cache, dim 4096) tensor-parallel over 8 NeuronCores.

Sharding (per core c): q heads [4c, 4c+4), kv head c; x replicated; each core
emits a [128, 32] column-chunked partial of its full-width [1, 4096] output
projection, summed + transposed host-side.

Schedule: three DMA queues (SP/ACT HWDGE, Pool SWDGE) each stream ONE mega
piece holding their share of {x/rot/id extras, wqkv, K^T, V} followed by their
share of the wo stream, balanced so all queues end together.  All attention
compute (q/k/v proj on the PE with [128,1] psum cols, RoPE via a host-built
block-diagonal rotation matmul, scores/exp/softmax-z, AV) runs while the wo
stream is still in flight, so the only exposed tail is: last wo block ->
4 matmuls -> psum drain -> one [128,32] f32 output DMA.

Output projection is TRANSPOSED on the PE: out^T[128 outs, 32 chunks] with
lhsT = wo block [128 contract-dim, 128 outs] and rhs = attn column [128, 1]
(free-dim-1 matmuls are ~free), accumulating 4 head blocks per out chunk.
The host undoes the [128, 32] chunk-major layout when summing partials.

Weights/KV move as fp16 (error ~1e-3 vs fp32 reference); matmul accumulation
is fp32 in PSUM; softmax statistics fp32.
"""

import numpy as np

import concourse.tile as tile
from concourse import bacc, mybir
from concourse.bass_utils import run_bass_kernel_spmd
from concourse.tile import add_dep_helper

N_CORES = 8
DIM = 4096
HEAD_DIM = 128
N_HEADS = 32
N_KV_HEADS = 8
REPEATS = N_HEADS // N_KV_HEADS  # 4 q heads per core
KV_LEN = 8192                    # start_pos + 1
KCH = DIM // 128                 # 32 contraction chunks
TCH = KV_LEN // 128              # 64 kv-position chunks
QCOLS = REPEATS * 128            # 512
WCOLS = QCOLS + 2 * 128          # 768 merged qkv cols per chunk
XTRA = KCH + 256                 # x (32) + rot (128) + id (128) cols
OCH = DIM // 128                 # 32 output col chunks
SCALE = 1.0 / np.sqrt(np.float32(HEAD_DIM))

F32 = mybir.dt.float32
F16 = mybir.dt.float16

# ---- stream split (tunable) -------------------------------------------------
# All queues stream [p1: wqkv+kt share][p2: v share][p3: wo share].  A DMA
# occupies its engine's whole instruction stream, so the exp (ACT) is chained
# between ACT's p2 and p3; ACT's p1+p2 are sized to end right as scores are
# ready so the exp never stalls the wo stream.  Queue 0 = SP (extras first,
# tiny final wo block + out DMA last), queue 1 = ACT (late start: act-table
# load), queue 2 = Pool.
W_SPLIT = [(0, 12), (12, 20), (20, 32)]
KT_SPLIT = [(45, 64), (15, 45), (0, 15)]   # chunk 63 (new k) on SP
V_SPLIT = [(41, 64), (34, 41), (0, 34)]    # chunk 63 (new v) on SP
# wo stream: 128 flat blocks of 128 cols in (oc, h) order.  Every queue
# streams [bulk][mid (2 oc groups)][last (1 oc group)] so each piece's
# matmul burst either pre-runs or is tiny; the drains split so only the
# late oc groups' drain sits in the exposed tail.
WO_BLOCKS = [
    list(range(62, 92)) + list(range(108, 116)) + list(range(124, 128)),
    list(range(20, 62)) + list(range(100, 108)) + list(range(120, 124)),
    list(range(0, 20)) + list(range(92, 100)) + list(range(116, 120)),
]
# per-queue piece cuts, in block counts (bulk, mid, last)
WO_CUTS = [(30, 8, 4), (42, 8, 4), (20, 8, 4)]
# proj accumulation order: chunks grouped by queue arrival (extras ride a
# tiny first DMA on Pool so x never gates the early waves)
PROJ_ORDER = (list(range(20, 32)) + list(range(12, 20)) + list(range(0, 12)))

_CACHED = {}


def _mega_layout():
    """Per-queue column layout of the stream tile: maps for wqkv chunk, kt
    chunk, v chunk -> (queue, col offset).  Extras live at the start of q0.
    Returns per-queue piece-1 boundary (piece 2 = trailing v cols)."""
    wq_off, kt_off, v_off, mega_cols, p1_cols = {}, {}, {}, [], []
    for q in range(3):
        off = XTRA if q == 2 else 0  # extras head q2 (own tiny first DMA)
        for c in range(*W_SPLIT[q]):
            wq_off[c] = (q, off)
            off += WCOLS
        for j in range(*KT_SPLIT[q]):
            kt_off[j] = (q, off)
            off += 128
        p1_cols.append(off)
        for j in range(*V_SPLIT[q]):
            v_off[j] = (q, off)
            off += 128
        mega_cols.append(off)
    return wq_off, kt_off, v_off, mega_cols, p1_cols


def _build():
    nc = bacc.Bacc(None, target_bir_lowering=False)

    wq_off, kt_off, v_off, mega_cols, p1_cols = _mega_layout()
    wo_cols = [len(bl) * 128 for bl in WO_BLOCKS]
    wo_pos = {b: (q, i) for q, bl in enumerate(WO_BLOCKS)
              for i, b in enumerate(bl)}

    s_d = [nc.dram_tensor(f"s{q}", [128, mega_cols[q]], F16, kind="ExternalInput")
           for q in range(3)]
    wo_d = [nc.dram_tensor(f"wo{q}", [128, wo_cols[q]], F16, kind="ExternalInput")
            for q in range(3)]
    out_p = nc.dram_tensor("out_p", [128, OCH], F32, kind="ExternalOutput")

    tails = [None, None, None]

    def chain(q, inst):
        if tails[q] is not None:
            add_dep_helper(inst.ins, tails[q].ins, sync=False,
                           reason="stream order")
        tails[q] = inst

    with tile.TileContext(nc) as tc:
        with (
            tc.tile_pool(name="big", bufs=1) as big,
            tc.tile_pool(name="small", bufs=1) as small,
        ):
            engs = [nc.sync, nc.scalar, nc.gpsimd]

            sb = [big.tile([128, mega_cols[q]], F16, name=f"sb{q}")
                  for q in range(3)]
            wo_sb = [big.tile([128, wo_cols[q]], F16, name=f"wosb{q}")
                     for q in range(3)]

            x_sb = sb[2][:, 0:KCH]
            rot_sb = sb[2][:, KCH:KCH + 128]
            id_sb = sb[2][:, KCH + 128:XTRA]

            def wblk(c, col):     # wqkv chunk c, inner col block [128]
                q, off = wq_off[c]
                return sb[q][:, off + col * 128: off + (col + 1) * 128]

            def ktblk(j):
                q, off = kt_off[j]
                return sb[q][:, off:off + 128]

            def vblk(j):
                q, off = v_off[j]
                return sb[q][:, off:off + 128]

            def woblk(oc, h):
                q, i = wo_pos[oc * REPEATS + h]
                return wo_sb[q][:, i * 128:(i + 1) * 128]

            qk_sb = small.tile([128, 6], F16)
            qT = small.tile([128, REPEATS], F16)
            attn = small.tile([128, REPEATS], F16)
            e_sb = small.tile([128, TCH * REPEATS], F16)
            zp_sb = small.tile([128, REPEATS], F32)
            rz_sb = small.tile([1, REPEATS], F32)
            rzb_sb = small.tile([128, REPEATS], F32)
            ones_sb = small.tile([128, 1], F32)
            ones_row = small.tile([1, 128], F32)
            o_sb = small.tile([128, OCH], F32)

            nc.vector.memset(ones_sb[:], 1.0)
            nc.vector.memset(ones_row[:], 1.0)

            # --- input streams: extras tiny-first on Pool, then per queue
            # [wqkv piece][kt piece][v piece] -- split pieces cost nothing
            # extra (chained DMAs pack back-to-back) and the earlier wqkv
            # sems let the projection waves pre-run ---
            chain(2, engs[2].dma_start(
                out=sb[2][:, 0:XTRA], in_=s_d[2][:, 0:XTRA]))
            wq_end = {q: (XTRA if q == 2 else 0) +
                      (W_SPLIT[q][1] - W_SPLIT[q][0]) * WCOLS for q in range(3)}
            for q in (0, 2, 1):
                lo = XTRA if q == 2 else 0
                chain(q, engs[q].dma_start(
                    out=sb[q][:, lo:wq_end[q]], in_=s_d[q][:, lo:wq_end[q]]))
            for q in (0, 2, 1):
                chain(q, engs[q].dma_start(
                    out=sb[q][:, wq_end[q]:p1_cols[q]],
                    in_=s_d[q][:, wq_end[q]:p1_cols[q]]))
            for q in (0, 2, 1):
                chain(q, engs[q].dma_start(
                    out=sb[q][:, p1_cols[q]:], in_=s_d[q][:, p1_cols[q]:]))

            with tc.tile_pool(name="ps", bufs=1, space="PSUM") as ps:
                pqkv = ps.tile([128, 6], F32)
                prot = ps.tile([128, 5], F32)
                pvrow = ps.tile([1, 128], F32)
                pscore = ps.tile([128, TCH * REPEATS], F32)
                pav = ps.tile([128, REPEATS], F32)
                pz = ps.tile([1, REPEATS], F32)
                przb = ps.tile([128, REPEATS], F32)
                pout = ps.tile([128, OCH], F32)

                # qkv projection, transposed: psum cols [q0 q1 q2 q3 k v];
                # chunks ordered by stream arrival so waves pre-run
                for col in range(6):
                    for i, c in enumerate(PROJ_ORDER):
                        nc.tensor.matmul(
                            pqkv[:, col:col + 1],
                            wblk(c, col),
                            x_sb[:, c:c + 1],
                            start=(i == 0), stop=(i == KCH - 1),
                        )
                nc.vector.tensor_copy(qk_sb[:], pqkv[:])
                # RoPE on q cols + k col in one matmul; v passes through
                nc.tensor.matmul(prot[:], rot_sb, qk_sb[:, 0:5],
                                 start=True, stop=True)
                nc.vector.tensor_copy(qT[:], prot[:, 0:REPEATS])
                # chunk 63's position slots are rotated host-side so the new
                # position (8191) sits at slot 0 -> col 0 of kt chunk 63
                nc.vector.tensor_copy(
                    ktblk(TCH - 1)[:, 0:1], prot[:, REPEATS:REPEATS + 1])
                # new-v row via identity matmul ([128,1] col -> [1,128] row)
                nc.tensor.matmul(pvrow[:], qk_sb[:, 5:6], id_sb,
                                 start=True, stop=True)

                # scores_T [128 t, 4 h] per chunk
                for j in range(TCH):
                    nc.tensor.matmul(
                        pscore[:, j * REPEATS:(j + 1) * REPEATS],
                        ktblk(j), qT[:], start=True, stop=True)
                # exp on ACT, chained between ACT's piece-1 and wo DMAs (a
                # DMA occupies its engine's whole stream, so the exp has to
                # sit here; ACT piece 1 is sized so the wait is short)
                ev = e_sb[:].rearrange("p (j h) -> p h j", h=REPEATS)
                chain(1, nc.scalar.activation(
                    e_sb[:], pscore[:],
                    mybir.ActivationFunctionType.Exp, scale=float(SCALE)))

                # --- wo streams (behind the exp on ACT): per-queue ladder
                # [bulk][mid][last] so matmul bursts pre-run ---
                for q in (2, 1, 0):
                    nb, nm, nl = WO_CUTS[q]
                    cuts = [0, nb * 128, (nb + nm) * 128, (nb + nm + nl) * 128]
                    for lo, hi in zip(cuts[:-1], cuts[1:]):
                        chain(q, engs[q].dma_start(
                            out=wo_sb[q][:, lo:hi], in_=wo_d[q][:, lo:hi]))

                # softmax z -> 1/z -> broadcast (DVE + PE, off the queues)
                nc.vector.reduce_sum(zp_sb[:], ev[:], axis=mybir.AxisListType.X)
                nc.tensor.matmul(pz[:], ones_sb[:], zp_sb[:],
                                 start=True, stop=True)
                nc.vector.reciprocal(rz_sb[:], pz[:])
                nc.tensor.matmul(przb[:], ones_row[:], rz_sb[:],
                                 start=True, stop=True)
                nc.vector.tensor_copy(rzb_sb[:], przb[:])
                # scatter new v into partition 0 of v chunk 63 (after its
                # piece lands; AV for chunk 63 runs last)
                vt = vblk(TCH - 1)
                nc.vector.tensor_copy(vt[0:1, :], pvrow[:])

                # AV; chunk 63 last (new-v row WAW)
                av_order = [j for j in range(TCH - 1)] + [TCH - 1]
                for idx, j in enumerate(av_order):
                    nc.tensor.matmul(
                        pav[:], vblk(j),
                        e_sb[:, j * REPEATS:(j + 1) * REPEATS],
                        start=(idx == 0), stop=(idx == TCH - 1),
                    )
                nc.vector.tensor_mul(attn[:], pav[:], rzb_sb[:])

                # transposed output projection: out^T[:, oc] accumulates 4
                # head blocks; free-dim-1 matmuls are ~free on the PE
                for oc in range(OCH):
                    for h in range(REPEATS):
                        nc.tensor.matmul(
                            pout[:, oc:oc + 1],
                            woblk(oc, h),
                            attn[:, h:h + 1],
                            start=(h == 0), stop=(h == REPEATS - 1),
                        )
                # split drain: bulk oc groups pre-run while the wo ladders'
                # mid/last blocks stream; only a 9-col drain sits in the tail
                nc.vector.tensor_copy(o_sb[:, 0:23], pout[:, 0:23])
                nc.vector.tensor_copy(o_sb[:, 23:], pout[:, 23:])
                chain(0, nc.sync.dma_start(out=out_p[:], in_=o_sb[:]))

    nc.compile()
    # Trim the program epilogue to just SP's completion waits (all DMA-queue
    # and engine sems, which include the output DMA) plus its drain; drop
    # both all-engine barrier rounds and the sem-reset ISA (~1us of pure
    # sem cascade).  Single-shot execution doesn't need the reset.
    end = nc.m.functions[0].blocks[-1]
    keep = []
    for inst in end.instructions:
        if inst.engine != mybir.EngineType.SP or isinstance(inst, mybir.InstDrain):
            continue
        si = inst.sync_info
        if si is None or not any(
                (w.ant_name or "").startswith("DMAHW0") for w in si.on_wait):
            continue
        keep.append(inst)
    assert keep, "expected an SP wait on its HWDGE queue sem"
    end.instructions = keep
    return nc


def _shard_inputs(x, wq, wk, wv, wo, cache_k, cache_v, cos, sin):
    """Build the 8 per-core input maps (fp16 weights/KV, C-contiguous)."""
    wq_off, kt_off, v_off, mega_cols, p1_cols = _mega_layout()

    x_flat = np.asarray(x, dtype=np.float32).reshape(DIM)
    x_col = x_flat.reshape(KCH, 128).T.astype(np.float16)  # [128, 32]

    cos = np.asarray(cos, np.float32).reshape(-1)  # [64]
    sin = np.asarray(sin, np.float32).reshape(-1)
    # rot = R.T (matmul lhsT layout) for the block-diag 2x2 rotation R
    rot = np.zeros((128, 128), np.float32)
    i = np.arange(64)
    rot[2 * i, 2 * i] = cos
    rot[2 * i + 1, 2 * i + 1] = cos
    rot[2 * i + 1, 2 * i] = -sin
    rot[2 * i, 2 * i + 1] = sin
    xtra = np.concatenate(
        [x_col, rot.astype(np.float16), np.eye(128, dtype=np.float16)], axis=1)

    wq = np.asarray(wq, np.float32)
    wk = np.asarray(wk, np.float32)
    wv = np.asarray(wv, np.float32)
    wo = np.asarray(wo, np.float32)
    cache_k = np.asarray(cache_k, np.float32)
    cache_v = np.asarray(cache_v, np.float32)

    in_maps = []
    for c in range(N_CORES):
        wq_c = wq[c * QCOLS:(c + 1) * QCOLS]              # [512, 4096]
        wk_c = wk[c * HEAD_DIM:(c + 1) * HEAD_DIM]        # [128, 4096]
        wv_c = wv[c * HEAD_DIM:(c + 1) * HEAD_DIM]
        q_blk = (wq_c.reshape(REPEATS, 128, KCH, 128)
                 .transpose(2, 3, 0, 1).reshape(KCH, 128, QCOLS))
        k_blk = wk_c.reshape(128, KCH, 128).transpose(1, 2, 0)
        v_blk = wv_c.reshape(128, KCH, 128).transpose(1, 2, 0)
        wqkv_c = np.concatenate([q_blk, k_blk, v_blk], axis=2)  # [32,128,768]
        wqkv_c = wqkv_c.astype(np.float16)
        # chunk 63 slot rotation: slot 0 <- new position (device-written),
        # slots 1..127 <- cache positions 8064..8190
        kraw = cache_k[0, :KV_LEN, c, :].T  # [128, 8192]
        k_c = np.empty((128, KV_LEN), np.float16)
        k_c[:, :KV_LEN - 128] = kraw[:, :KV_LEN - 128]
        k_c[:, KV_LEN - 128] = 0
        k_c[:, KV_LEN - 127:] = kraw[:, KV_LEN - 128:KV_LEN - 1]
        vraw = cache_v[0, :KV_LEN, c, :]  # [8192, 128]
        v_c = np.empty((TCH, 128, HEAD_DIM), np.float16)
        v_c[:TCH - 1] = vraw[:KV_LEN - 128].reshape(TCH - 1, 128, HEAD_DIM)
        v_c[TCH - 1, 0] = 0
        v_c[TCH - 1, 1:] = vraw[KV_LEN - 128:KV_LEN - 1]
        v_c = v_c.transpose(1, 0, 2)  # [128, 64, 128]

        m = {}
        for q in range(3):
            parts = []
            if q == 2:
                parts.append(xtra)
            for cc in range(*W_SPLIT[q]):
                parts.append(wqkv_c[cc])
            lo, hi = KT_SPLIT[q]
            parts.append(k_c[:, lo * 128:hi * 128])
            lo, hi = V_SPLIT[q]
            parts.append(v_c[:, lo:hi].reshape(128, (hi - lo) * 128))
            m[f"s{q}"] = np.ascontiguousarray(np.concatenate(parts, axis=1))
            assert m[f"s{q}"].shape[1] == mega_cols[q]
        wo_c = wo[:, c * QCOLS:(c + 1) * QCOLS].astype(np.float16)  # [4096,512]
        for q, bl in enumerate(WO_BLOCKS):
            blocks = []
            for b in bl:
                oc, h = b // REPEATS, b % REPEATS
                blocks.append(
                    wo_c[oc * 128:(oc + 1) * 128, h * 128:(h + 1) * 128].T)
            m[f"wo{q}"] = np.ascontiguousarray(np.concatenate(blocks, axis=1))
        in_maps.append(m)
    return in_maps


def get_program(reps=1):
    if "nc" not in _CACHED:
        _CACHED["nc"] = _build()
    return _CACHED["nc"]


def kernel(x, wq, wk, wv, wo, cache_k, cache_v, cos, sin, start_pos):
    nc = get_program()
    in_maps = _shard_inputs(x, wq, wk, wv, wo, cache_k, cache_v, cos, sin)
    res = run_bass_kernel_spmd(nc, in_maps, list(range(N_CORES)))
    out = np.zeros(DIM, np.float32)
    for c in range(N_CORES):
        out += res.results[c]["out_p"].T.reshape(DIM)
    return out.reshape(1, 1, DIM)


# revision 37
# speedup vs baseline: 1.0912x; 1.0912x over previous
"""Single-token GQA decode attention (32 q heads / 8 kv heads, 8192-pos KV
cache, dim 4096) tensor-parallel over 8 NeuronCores.

Sharding (per core c): q heads [4c, 4c+4), kv head c; x replicated; each core
emits a [128, 32] column-chunked partial of its full-width [1, 4096] output
projection, summed + transposed host-side.

Schedule: three DMA queues (SP/ACT HWDGE, Pool SWDGE) each stream ONE mega
piece holding their share of {x/rot/id extras, wqkv, K^T, V} followed by their
share of the wo stream, balanced so all queues end together.  All attention
compute (q/k/v proj on the PE with [128,1] psum cols, RoPE via a host-built
block-diagonal rotation matmul, scores/exp/softmax-z, AV) runs while the wo
stream is still in flight, so the only exposed tail is: last wo block ->
4 matmuls -> psum drain -> one [128,32] f32 output DMA.

Output projection is TRANSPOSED on the PE: out^T[128 outs, 32 chunks] with
lhsT = wo block [128 contract-dim, 128 outs] and rhs = attn column [128, 1]
(free-dim-1 matmuls are ~free), accumulating 4 head blocks per out chunk.
The host undoes the [128, 32] chunk-major layout when summing partials.

Weights/KV move as fp16 (error ~1e-3 vs fp32 reference); matmul accumulation
is fp32 in PSUM; softmax statistics fp32.
"""

import numpy as np

import concourse.tile as tile
from concourse import bacc, mybir
from concourse.bass_utils import run_bass_kernel_spmd
from concourse.tile import add_dep_helper

N_CORES = 8
DIM = 4096
HEAD_DIM = 128
N_HEADS = 32
N_KV_HEADS = 8
REPEATS = N_HEADS // N_KV_HEADS  # 4 q heads per core
KV_LEN = 8192                    # start_pos + 1
KCH = DIM // 128                 # 32 contraction chunks
TCH = KV_LEN // 128              # 64 kv-position chunks
QCOLS = REPEATS * 128            # 512
WCOLS = QCOLS + 2 * 128          # 768 merged qkv cols per chunk
XTRA = KCH + 256                 # x (32) + rot (128) + id (128) cols
OCH = DIM // 128                 # 32 output col chunks
SCALE = 1.0 / np.sqrt(np.float32(HEAD_DIM))

F32 = mybir.dt.float32
F16 = mybir.dt.float16

# ---- stream split (tunable) -------------------------------------------------
# All queues stream [p1: wqkv+kt share][p2: v share][p3: wo share].  A DMA
# occupies its engine's whole instruction stream, so the exp (ACT) is chained
# between ACT's p2 and p3; ACT's p1+p2 are sized to end right as scores are
# ready so the exp never stalls the wo stream.  Queue 0 = SP (extras first,
# tiny final wo block + out DMA last), queue 1 = ACT (late start: act-table
# load), queue 2 = Pool.
W_SPLIT = [(0, 12), (12, 20), (20, 32)]
KT_SPLIT = [(45, 64), (15, 45), (0, 15)]   # chunk 63 (new k) on SP
V_SPLIT = [(41, 64), (27, 41), (0, 27)]    # chunk 63 (new v) on SP
# wo stream: 128 flat blocks of 128 cols in (oc, h) order.  Every queue
# streams [bulk][mid (2 oc groups)][last (1 oc group)] so each piece's
# matmul burst either pre-runs or is tiny; the drains split so only the
# late oc groups' drain sits in the exposed tail.
WO_BLOCKS = [
    list(range(62, 92)) + list(range(108, 116)) + list(range(124, 128)),
    list(range(27, 62)) + list(range(100, 108)) + list(range(120, 124)),
    list(range(0, 27)) + list(range(92, 100)) + list(range(116, 120)),
]
# per-queue piece cuts, in block counts (bulk, mid, last)
WO_CUTS = [(30, 8, 4), (35, 8, 4), (27, 8, 4)]
# proj accumulation order: chunks grouped by queue arrival (extras ride a
# tiny first DMA on Pool so x never gates the early waves)
PROJ_ORDER = (list(range(20, 32)) + list(range(12, 20)) + list(range(0, 12)))

_CACHED = {}


def _mega_layout():
    """Per-queue column layout of the stream tile: maps for wqkv chunk, kt
    chunk, v chunk -> (queue, col offset).  Extras live at the start of q0.
    Returns per-queue piece-1 boundary (piece 2 = trailing v cols)."""
    wq_off, kt_off, v_off, mega_cols, p1_cols = {}, {}, {}, [], []
    for q in range(3):
        off = XTRA if q == 2 else 0  # extras head q2 (own tiny first DMA)
        for c in range(*W_SPLIT[q]):
            wq_off[c] = (q, off)
            off += WCOLS
        for j in range(*KT_SPLIT[q]):
            kt_off[j] = (q, off)
            off += 128
        p1_cols.append(off)
        for j in range(*V_SPLIT[q]):
            v_off[j] = (q, off)
            off += 128
        mega_cols.append(off)
    return wq_off, kt_off, v_off, mega_cols, p1_cols


def _build():
    nc = bacc.Bacc(None, target_bir_lowering=False)

    wq_off, kt_off, v_off, mega_cols, p1_cols = _mega_layout()
    wo_cols = [len(bl) * 128 for bl in WO_BLOCKS]
    wo_pos = {b: (q, i) for q, bl in enumerate(WO_BLOCKS)
              for i, b in enumerate(bl)}

    s_d = [nc.dram_tensor(f"s{q}", [128, mega_cols[q]], F16, kind="ExternalInput")
           for q in range(3)]
    wo_d = [nc.dram_tensor(f"wo{q}", [128, wo_cols[q]], F16, kind="ExternalInput")
            for q in range(3)]
    out_p = nc.dram_tensor("out_p", [128, OCH], F32, kind="ExternalOutput")

    tails = [None, None, None]

    def chain(q, inst):
        if tails[q] is not None:
            add_dep_helper(inst.ins, tails[q].ins, sync=False,
                           reason="stream order")
        tails[q] = inst

    with tile.TileContext(nc) as tc:
        with (
            tc.tile_pool(name="big", bufs=1) as big,
            tc.tile_pool(name="small", bufs=1) as small,
        ):
            engs = [nc.sync, nc.scalar, nc.gpsimd]

            sb = [big.tile([128, mega_cols[q]], F16, name=f"sb{q}")
                  for q in range(3)]
            wo_sb = [big.tile([128, wo_cols[q]], F16, name=f"wosb{q}")
                     for q in range(3)]

            x_sb = sb[2][:, 0:KCH]
            rot_sb = sb[2][:, KCH:KCH + 128]
            id_sb = sb[2][:, KCH + 128:XTRA]

            def wblk(c, col):     # wqkv chunk c, inner col block [128]
                q, off = wq_off[c]
                return sb[q][:, off + col * 128: off + (col + 1) * 128]

            def ktblk(j):
                q, off = kt_off[j]
                return sb[q][:, off:off + 128]

            def vblk(j):
                q, off = v_off[j]
                return sb[q][:, off:off + 128]

            def woblk(oc, h):
                q, i = wo_pos[oc * REPEATS + h]
                return wo_sb[q][:, i * 128:(i + 1) * 128]

            qk_sb = small.tile([128, 6], F16)
            qT = small.tile([128, REPEATS], F16)
            attn = small.tile([128, REPEATS], F16)
            e_sb = small.tile([128, TCH * REPEATS], F16)
            zp_sb = small.tile([128, REPEATS], F32)
            rz_sb = small.tile([1, REPEATS], F32)
            rzb_sb = small.tile([128, REPEATS], F32)
            ones_sb = small.tile([128, 1], F32)
            ones_row = small.tile([1, 128], F32)
            o_sb = small.tile([128, OCH], F32)

            nc.vector.memset(ones_sb[:], 1.0)
            nc.vector.memset(ones_row[:], 1.0)

            # --- input streams: extras tiny-first on Pool, then per queue
            # [wqkv piece][kt piece][v piece] -- split pieces cost nothing
            # extra (chained DMAs pack back-to-back) and the earlier wqkv
            # sems let the projection waves pre-run ---
            chain(2, engs[2].dma_start(
                out=sb[2][:, 0:XTRA], in_=s_d[2][:, 0:XTRA]))
            wq_end = {q: (XTRA if q == 2 else 0) +
                      (W_SPLIT[q][1] - W_SPLIT[q][0]) * WCOLS for q in range(3)}
            for q in (0, 2, 1):
                lo = XTRA if q == 2 else 0
                chain(q, engs[q].dma_start(
                    out=sb[q][:, lo:wq_end[q]], in_=s_d[q][:, lo:wq_end[q]]))
            for q in (0, 2, 1):
                chain(q, engs[q].dma_start(
                    out=sb[q][:, wq_end[q]:p1_cols[q]],
                    in_=s_d[q][:, wq_end[q]:p1_cols[q]]))
            for q in (0, 2, 1):
                chain(q, engs[q].dma_start(
                    out=sb[q][:, p1_cols[q]:], in_=s_d[q][:, p1_cols[q]:]))

            with tc.tile_pool(name="ps", bufs=1, space="PSUM") as ps:
                pqkv = ps.tile([128, 6], F32)
                prot = ps.tile([128, 5], F32)
                pvrow = ps.tile([1, 128], F32)
                pscore = ps.tile([128, TCH * REPEATS], F32)
                pav = ps.tile([128, REPEATS], F32)
                pz = ps.tile([1, REPEATS], F32)
                przb = ps.tile([128, REPEATS], F32)
                pout = ps.tile([128, OCH], F32)

                # qkv projection, transposed: psum cols [q0 q1 q2 q3 k v];
                # chunks ordered by stream arrival so waves pre-run
                for col in range(6):
                    for i, c in enumerate(PROJ_ORDER):
                        nc.tensor.matmul(
                            pqkv[:, col:col + 1],
                            wblk(c, col),
                            x_sb[:, c:c + 1],
                            start=(i == 0), stop=(i == KCH - 1),
                        )
                nc.vector.tensor_copy(qk_sb[:], pqkv[:])
                # RoPE on q cols + k col in one matmul; v passes through
                nc.tensor.matmul(prot[:], rot_sb, qk_sb[:, 0:5],
                                 start=True, stop=True)
                nc.vector.tensor_copy(qT[:], prot[:, 0:REPEATS])
                # chunk 63's position slots are rotated host-side so the new
                # position (8191) sits at slot 0 -> col 0 of kt chunk 63
                nc.vector.tensor_copy(
                    ktblk(TCH - 1)[:, 0:1], prot[:, REPEATS:REPEATS + 1])
                # new-v row via identity matmul ([128,1] col -> [1,128] row)
                nc.tensor.matmul(pvrow[:], qk_sb[:, 5:6], id_sb,
                                 start=True, stop=True)

                # scores_T [128 t, 4 h] per chunk
                for j in range(TCH):
                    nc.tensor.matmul(
                        pscore[:, j * REPEATS:(j + 1) * REPEATS],
                        ktblk(j), qT[:], start=True, stop=True)
                # exp on ACT, chained between ACT's piece-1 and wo DMAs (a
                # DMA occupies its engine's whole stream, so the exp has to
                # sit here; ACT piece 1 is sized so the wait is short)
                ev = e_sb[:].rearrange("p (j h) -> p h j", h=REPEATS)
                chain(1, nc.scalar.activation(
                    e_sb[:], pscore[:],
                    mybir.ActivationFunctionType.Exp, scale=float(SCALE)))

                # --- wo streams (behind the exp on ACT): per-queue ladder
                # [bulk][mid][last] so matmul bursts pre-run ---
                for q in (2, 1, 0):
                    nb, nm, nl = WO_CUTS[q]
                    cuts = [0, nb * 128, (nb + nm) * 128, (nb + nm + nl) * 128]
                    for lo, hi in zip(cuts[:-1], cuts[1:]):
                        chain(q, engs[q].dma_start(
                            out=wo_sb[q][:, lo:hi], in_=wo_d[q][:, lo:hi]))

                # softmax z -> 1/z -> broadcast (DVE + PE, off the queues)
                nc.vector.reduce_sum(zp_sb[:], ev[:], axis=mybir.AxisListType.X)
                nc.tensor.matmul(pz[:], ones_sb[:], zp_sb[:],
                                 start=True, stop=True)
                nc.vector.reciprocal(rz_sb[:], pz[:])
                nc.tensor.matmul(przb[:], ones_row[:], rz_sb[:],
                                 start=True, stop=True)
                nc.vector.tensor_copy(rzb_sb[:], przb[:])
                # scatter new v into partition 0 of v chunk 63 (after its
                # piece lands; AV for chunk 63 runs last)
                vt = vblk(TCH - 1)
                nc.vector.tensor_copy(vt[0:1, :], pvrow[:])

                # AV; chunk 63 last (new-v row WAW)
                av_order = [j for j in range(TCH - 1)] + [TCH - 1]
                for idx, j in enumerate(av_order):
                    nc.tensor.matmul(
                        pav[:], vblk(j),
                        e_sb[:, j * REPEATS:(j + 1) * REPEATS],
                        start=(idx == 0), stop=(idx == TCH - 1),
                    )
                nc.vector.tensor_mul(attn[:], pav[:], rzb_sb[:])

                # transposed output projection: out^T[:, oc] accumulates 4
                # head blocks; free-dim-1 matmuls are ~free on the PE
                for oc in range(OCH):
                    for h in range(REPEATS):
                        nc.tensor.matmul(
                            pout[:, oc:oc + 1],
                            woblk(oc, h),
                            attn[:, h:h + 1],
                            start=(h == 0), stop=(h == REPEATS - 1),
                        )
                # split drain: bulk oc groups pre-run while the wo ladders'
                # mid/last blocks stream; only a 9-col drain sits in the tail
                nc.vector.tensor_copy(o_sb[:, 0:23], pout[:, 0:23])
                nc.vector.tensor_copy(o_sb[:, 23:], pout[:, 23:])
                chain(0, nc.sync.dma_start(out=out_p[:], in_=o_sb[:]))

    nc.compile()
    # Trim the program epilogue to just SP's completion waits (all DMA-queue
    # and engine sems, which include the output DMA) plus its drain; drop
    # both all-engine barrier rounds and the sem-reset ISA (~1us of pure
    # sem cascade).  Single-shot execution doesn't need the reset.
    end = nc.m.functions[0].blocks[-1]
    keep = []
    for inst in end.instructions:
        if inst.engine != mybir.EngineType.SP or isinstance(inst, mybir.InstDrain):
            continue
        si = inst.sync_info
        if si is None or not any(
                (w.ant_name or "").startswith("DMAHW0") for w in si.on_wait):
            continue
        keep.append(inst)
    assert keep, "expected an SP wait on its HWDGE queue sem"
    end.instructions = keep
    return nc


def _shard_inputs(x, wq, wk, wv, wo, cache_k, cache_v, cos, sin):
    """Build the 8 per-core input maps (fp16 weights/KV, C-contiguous)."""
    wq_off, kt_off, v_off, mega_cols, p1_cols = _mega_layout()

    x_flat = np.asarray(x, dtype=np.float32).reshape(DIM)
    x_col = x_flat.reshape(KCH, 128).T.astype(np.float16)  # [128, 32]

    cos = np.asarray(cos, np.float32).reshape(-1)  # [64]
    sin = np.asarray(sin, np.float32).reshape(-1)
    # rot = R.T (matmul lhsT layout) for the block-diag 2x2 rotation R
    rot = np.zeros((128, 128), np.float32)
    i = np.arange(64)
    rot[2 * i, 2 * i] = cos
    rot[2 * i + 1, 2 * i + 1] = cos
    rot[2 * i + 1, 2 * i] = -sin
    rot[2 * i, 2 * i + 1] = sin
    xtra = np.concatenate(
        [x_col, rot.astype(np.float16), np.eye(128, dtype=np.float16)], axis=1)

    wq = np.asarray(wq, np.float32)
    wk = np.asarray(wk, np.float32)
    wv = np.asarray(wv, np.float32)
    wo = np.asarray(wo, np.float32)
    cache_k = np.asarray(cache_k, np.float32)
    cache_v = np.asarray(cache_v, np.float32)

    in_maps = []
    for c in range(N_CORES):
        wq_c = wq[c * QCOLS:(c + 1) * QCOLS]              # [512, 4096]
        wk_c = wk[c * HEAD_DIM:(c + 1) * HEAD_DIM]        # [128, 4096]
        wv_c = wv[c * HEAD_DIM:(c + 1) * HEAD_DIM]
        q_blk = (wq_c.reshape(REPEATS, 128, KCH, 128)
                 .transpose(2, 3, 0, 1).reshape(KCH, 128, QCOLS))
        k_blk = wk_c.reshape(128, KCH, 128).transpose(1, 2, 0)
        v_blk = wv_c.reshape(128, KCH, 128).transpose(1, 2, 0)
        wqkv_c = np.concatenate([q_blk, k_blk, v_blk], axis=2)  # [32,128,768]
        wqkv_c = wqkv_c.astype(np.float16)
        # chunk 63 slot rotation: slot 0 <- new position (device-written),
        # slots 1..127 <- cache positions 8064..8190
        kraw = cache_k[0, :KV_LEN, c, :].T  # [128, 8192]
        k_c = np.empty((128, KV_LEN), np.float16)
        k_c[:, :KV_LEN - 128] = kraw[:, :KV_LEN - 128]
        k_c[:, KV_LEN - 128] = 0
        k_c[:, KV_LEN - 127:] = kraw[:, KV_LEN - 128:KV_LEN - 1]
        vraw = cache_v[0, :KV_LEN, c, :]  # [8192, 128]
        v_c = np.empty((TCH, 128, HEAD_DIM), np.float16)
        v_c[:TCH - 1] = vraw[:KV_LEN - 128].reshape(TCH - 1, 128, HEAD_DIM)
        v_c[TCH - 1, 0] = 0
        v_c[TCH - 1, 1:] = vraw[KV_LEN - 128:KV_LEN - 1]
        v_c = v_c.transpose(1, 0, 2)  # [128, 64, 128]

        m = {}
        for q in range(3):
            parts = []
            if q == 2:
                parts.append(xtra)
            for cc in range(*W_SPLIT[q]):
                parts.append(wqkv_c[cc])
            lo, hi = KT_SPLIT[q]
            parts.append(k_c[:, lo * 128:hi * 128])
            lo, hi = V_SPLIT[q]
            parts.append(v_c[:, lo:hi].reshape(128, (hi - lo) * 128))
            m[f"s{q}"] = np.ascontiguousarray(np.concatenate(parts, axis=1))
            assert m[f"s{q}"].shape[1] == mega_cols[q]
        wo_c = wo[:, c * QCOLS:(c + 1) * QCOLS].astype(np.float16)  # [4096,512]
        for q, bl in enumerate(WO_BLOCKS):
            blocks = []
            for b in bl:
                oc, h = b // REPEATS, b % REPEATS
                blocks.append(
                    wo_c[oc * 128:(oc + 1) * 128, h * 128:(h + 1) * 128].T)
            m[f"wo{q}"] = np.ascontiguousarray(np.concatenate(blocks, axis=1))
        in_maps.append(m)
    return in_maps


def get_program(reps=1):
    if "nc" not in _CACHED:
        _CACHED["nc"] = _build()
    return _CACHED["nc"]


def kernel(x, wq, wk, wv, wo, cache_k, cache_v, cos, sin, start_pos):
    nc = get_program()
    in_maps = _shard_inputs(x, wq, wk, wv, wo, cache_k, cache_v, cos, sin)
    res = run_bass_kernel_spmd(nc, in_maps, list(range(N_CORES)))
    out = np.zeros(DIM, np.float32)
    for c in range(N_CORES):
        out += res.results[c]["out_p"].T.reshape(DIM)
    return out.reshape(1, 1, DIM)


# revision 39
# speedup vs baseline: 1.1015x; 1.0095x over previous
"""Single-token GQA decode attention (32 q heads / 8 kv heads, 8192-pos KV
cache, dim 4096) tensor-parallel over 8 NeuronCores.

Sharding (per core c): q heads [4c, 4c+4), kv head c; x replicated; each core
emits a [128, 32] column-chunked partial of its full-width [1, 4096] output
projection, summed + transposed host-side.

Schedule: three DMA queues (SP/ACT HWDGE, Pool SWDGE) each stream ONE mega
piece holding their share of {x/rot/id extras, wqkv, K^T, V} followed by their
share of the wo stream, balanced so all queues end together.  All attention
compute (q/k/v proj on the PE with [128,1] psum cols, RoPE via a host-built
block-diagonal rotation matmul, scores/exp/softmax-z, AV) runs while the wo
stream is still in flight, so the only exposed tail is: last wo block ->
4 matmuls -> psum drain -> one [128,32] f32 output DMA.

Output projection is TRANSPOSED on the PE: out^T[128 outs, 32 chunks] with
lhsT = wo block [128 contract-dim, 128 outs] and rhs = attn column [128, 1]
(free-dim-1 matmuls are ~free), accumulating 4 head blocks per out chunk.
The host undoes the [128, 32] chunk-major layout when summing partials.

Weights/KV move as fp16 (error ~1e-3 vs fp32 reference); matmul accumulation
is fp32 in PSUM; softmax statistics fp32.
"""

import numpy as np

import concourse.tile as tile
from concourse import bacc, mybir
from concourse.bass_utils import run_bass_kernel_spmd
from concourse.tile import add_dep_helper

N_CORES = 8
DIM = 4096
HEAD_DIM = 128
N_HEADS = 32
N_KV_HEADS = 8
REPEATS = N_HEADS // N_KV_HEADS  # 4 q heads per core
KV_LEN = 8192                    # start_pos + 1
KCH = DIM // 128                 # 32 contraction chunks
TCH = KV_LEN // 128              # 64 kv-position chunks
QCOLS = REPEATS * 128            # 512
WCOLS = QCOLS + 2 * 128          # 768 merged qkv cols per chunk
XTRA = KCH + 256                 # x (32) + rot (128) + id (128) cols
OCH = DIM // 128                 # 32 output col chunks
SCALE = 1.0 / np.sqrt(np.float32(HEAD_DIM))

F32 = mybir.dt.float32
F16 = mybir.dt.float16

# ---- stream split (tunable) -------------------------------------------------
# All queues stream [p1: wqkv+kt share][p2: v share][p3: wo share].  A DMA
# occupies its engine's whole instruction stream, so the exp (ACT) is chained
# between ACT's p2 and p3; ACT's p1+p2 are sized to end right as scores are
# ready so the exp never stalls the wo stream.  Queue 0 = SP (extras first,
# tiny final wo block + out DMA last), queue 1 = ACT (late start: act-table
# load), queue 2 = Pool.
W_SPLIT = [(0, 12), (12, 20), (20, 32)]
KT_SPLIT = [(45, 64), (15, 45), (0, 15)]   # chunk 63 (new k) on SP
V_SPLIT = [(41, 64), (27, 41), (0, 27)]    # chunk 63 (new v) on SP
# wo stream: 128 flat blocks of 128 cols in (oc, h) order.  Every queue
# streams [bulk][mid (2 oc groups)][last (1 oc group)] so each piece's
# matmul burst either pre-runs or is tiny; the drains split so only the
# late oc groups' drain sits in the exposed tail.
WO_BLOCKS = [
    list(range(62, 92)) + list(range(104, 110)) + list(range(122, 128)),
    list(range(28, 62)) + list(range(98, 104)) + list(range(116, 122)),
    list(range(0, 28)) + list(range(92, 98)) + list(range(110, 116)),
]
# per-queue piece cuts, in block counts (bulk, mid, last)
WO_CUTS = [(30, 6, 6), (34, 6, 6), (28, 6, 6)]
# proj accumulation order: chunks grouped by queue arrival (extras ride a
# tiny first DMA on Pool so x never gates the early waves)
PROJ_ORDER = (list(range(20, 32)) + list(range(12, 20)) + list(range(0, 12)))

_CACHED = {}


def _mega_layout():
    """Per-queue column layout of the stream tile: maps for wqkv chunk, kt
    chunk, v chunk -> (queue, col offset).  Extras live at the start of q0.
    Returns per-queue piece-1 boundary (piece 2 = trailing v cols)."""
    wq_off, kt_off, v_off, mega_cols, p1_cols = {}, {}, {}, [], []
    for q in range(3):
        off = XTRA if q == 2 else 0  # extras head q2 (own tiny first DMA)
        for c in range(*W_SPLIT[q]):
            wq_off[c] = (q, off)
            off += WCOLS
        for j in range(*KT_SPLIT[q]):
            kt_off[j] = (q, off)
            off += 128
        p1_cols.append(off)
        for j in range(*V_SPLIT[q]):
            v_off[j] = (q, off)
            off += 128
        mega_cols.append(off)
    return wq_off, kt_off, v_off, mega_cols, p1_cols


def _build():
    nc = bacc.Bacc(None, target_bir_lowering=False)

    wq_off, kt_off, v_off, mega_cols, p1_cols = _mega_layout()
    wo_cols = [len(bl) * 128 for bl in WO_BLOCKS]
    wo_pos = {b: (q, i) for q, bl in enumerate(WO_BLOCKS)
              for i, b in enumerate(bl)}

    s_d = [nc.dram_tensor(f"s{q}", [128, mega_cols[q]], F16, kind="ExternalInput")
           for q in range(3)]
    wo_d = [nc.dram_tensor(f"wo{q}", [128, wo_cols[q]], F16, kind="ExternalInput")
            for q in range(3)]
    out_p = nc.dram_tensor("out_p", [128, OCH], F32, kind="ExternalOutput")

    tails = [None, None, None]

    def chain(q, inst):
        if tails[q] is not None:
            add_dep_helper(inst.ins, tails[q].ins, sync=False,
                           reason="stream order")
        tails[q] = inst

    with tile.TileContext(nc) as tc:
        with (
            tc.tile_pool(name="big", bufs=1) as big,
            tc.tile_pool(name="small", bufs=1) as small,
        ):
            engs = [nc.sync, nc.scalar, nc.gpsimd]

            sb = [big.tile([128, mega_cols[q]], F16, name=f"sb{q}")
                  for q in range(3)]
            wo_sb = [big.tile([128, wo_cols[q]], F16, name=f"wosb{q}")
                     for q in range(3)]

            x_sb = sb[2][:, 0:KCH]
            rot_sb = sb[2][:, KCH:KCH + 128]
            id_sb = sb[2][:, KCH + 128:XTRA]

            def wblk(c, col):     # wqkv chunk c, inner col block [128]
                q, off = wq_off[c]
                return sb[q][:, off + col * 128: off + (col + 1) * 128]

            def ktblk(j):
                q, off = kt_off[j]
                return sb[q][:, off:off + 128]

            def vblk(j):
                q, off = v_off[j]
                return sb[q][:, off:off + 128]

            def woblk(oc, h):
                q, i = wo_pos[oc * REPEATS + h]
                return wo_sb[q][:, i * 128:(i + 1) * 128]

            qk_sb = small.tile([128, 6], F16)
            qT = small.tile([128, REPEATS], F16)
            attn = small.tile([128, REPEATS], F16)
            e_sb = small.tile([128, TCH * REPEATS], F16)
            zp_sb = small.tile([128, REPEATS], F32)
            rz_sb = small.tile([1, REPEATS], F32)
            rzb_sb = small.tile([128, REPEATS], F32)
            ones_sb = small.tile([128, 1], F32)
            ones_row = small.tile([1, 128], F32)
            o_sb = small.tile([128, OCH], F32)

            nc.vector.memset(ones_sb[:], 1.0)
            nc.vector.memset(ones_row[:], 1.0)

            # --- input streams: per queue [wqkv piece (extras head q2)]
            # [kt piece][v piece] -- split pieces cost nothing extra
            # (chained DMAs pack back-to-back) and the earlier wqkv sems
            # let the projection waves pre-run ---
            wq_end = {q: (XTRA if q == 2 else 0) +
                      (W_SPLIT[q][1] - W_SPLIT[q][0]) * WCOLS for q in range(3)}
            for q in (0, 2, 1):
                chain(q, engs[q].dma_start(
                    out=sb[q][:, 0:wq_end[q]], in_=s_d[q][:, 0:wq_end[q]]))
            for q in (0, 2, 1):
                chain(q, engs[q].dma_start(
                    out=sb[q][:, wq_end[q]:p1_cols[q]],
                    in_=s_d[q][:, wq_end[q]:p1_cols[q]]))
            for q in (0, 2, 1):
                chain(q, engs[q].dma_start(
                    out=sb[q][:, p1_cols[q]:], in_=s_d[q][:, p1_cols[q]:]))

            with tc.tile_pool(name="ps", bufs=1, space="PSUM") as ps:
                pqkv = ps.tile([128, 6], F32)
                prot = ps.tile([128, 5], F32)
                pvrow = ps.tile([1, 128], F32)
                pscore = ps.tile([128, TCH * REPEATS], F32)
                pav = ps.tile([128, REPEATS], F32)
                pz = ps.tile([1, REPEATS], F32)
                przb = ps.tile([128, REPEATS], F32)
                pout = ps.tile([128, OCH], F32)

                # qkv projection, transposed: psum cols [q0 q1 q2 q3 k v];
                # chunks ordered by stream arrival so waves pre-run
                for col in range(6):
                    for i, c in enumerate(PROJ_ORDER):
                        nc.tensor.matmul(
                            pqkv[:, col:col + 1],
                            wblk(c, col),
                            x_sb[:, c:c + 1],
                            start=(i == 0), stop=(i == KCH - 1),
                        )
                nc.vector.tensor_copy(qk_sb[:], pqkv[:])
                # RoPE on q cols + k col in one matmul; v passes through
                nc.tensor.matmul(prot[:], rot_sb, qk_sb[:, 0:5],
                                 start=True, stop=True)
                nc.vector.tensor_copy(qT[:], prot[:, 0:REPEATS])
                # chunk 63's position slots are rotated host-side so the new
                # position (8191) sits at slot 0 -> col 0 of kt chunk 63
                nc.vector.tensor_copy(
                    ktblk(TCH - 1)[:, 0:1], prot[:, REPEATS:REPEATS + 1])
                # new-v row via identity matmul ([128,1] col -> [1,128] row)
                nc.tensor.matmul(pvrow[:], qk_sb[:, 5:6], id_sb,
                                 start=True, stop=True)

                # scores_T [128 t, 4 h] per chunk
                for j in range(TCH):
                    nc.tensor.matmul(
                        pscore[:, j * REPEATS:(j + 1) * REPEATS],
                        ktblk(j), qT[:], start=True, stop=True)
                # exp on ACT, chained between ACT's piece-1 and wo DMAs (a
                # DMA occupies its engine's whole stream, so the exp has to
                # sit here; ACT piece 1 is sized so the wait is short)
                ev = e_sb[:].rearrange("p (j h) -> p h j", h=REPEATS)
                chain(1, nc.scalar.activation(
                    e_sb[:], pscore[:],
                    mybir.ActivationFunctionType.Exp, scale=float(SCALE)))

                # --- wo streams (behind the exp on ACT): per-queue ladder
                # [bulk][mid][last] so matmul bursts pre-run ---
                for q in (2, 1, 0):
                    nb, nm, nl = WO_CUTS[q]
                    cuts = [0, nb * 128, (nb + nm) * 128, (nb + nm + nl) * 128]
                    for lo, hi in zip(cuts[:-1], cuts[1:]):
                        chain(q, engs[q].dma_start(
                            out=wo_sb[q][:, lo:hi], in_=wo_d[q][:, lo:hi]))

                # softmax z -> 1/z -> broadcast (DVE + PE, off the queues)
                nc.vector.reduce_sum(zp_sb[:], ev[:], axis=mybir.AxisListType.X)
                nc.tensor.matmul(pz[:], ones_sb[:], zp_sb[:],
                                 start=True, stop=True)
                nc.vector.reciprocal(rz_sb[:], pz[:])
                nc.tensor.matmul(przb[:], ones_row[:], rz_sb[:],
                                 start=True, stop=True)
                nc.vector.tensor_copy(rzb_sb[:], przb[:])
                # scatter new v into partition 0 of v chunk 63 (after its
                # piece lands; AV for chunk 63 runs last)
                vt = vblk(TCH - 1)
                nc.vector.tensor_copy(vt[0:1, :], pvrow[:])

                # AV; chunk 63 last (new-v row WAW)
                av_order = [j for j in range(TCH - 1)] + [TCH - 1]
                for idx, j in enumerate(av_order):
                    nc.tensor.matmul(
                        pav[:], vblk(j),
                        e_sb[:, j * REPEATS:(j + 1) * REPEATS],
                        start=(idx == 0), stop=(idx == TCH - 1),
                    )
                nc.vector.tensor_mul(attn[:], pav[:], rzb_sb[:])

                # transposed output projection: out^T[:, oc] accumulates 4
                # head blocks; free-dim-1 matmuls are ~free on the PE
                for oc in range(OCH):
                    for h in range(REPEATS):
                        nc.tensor.matmul(
                            pout[:, oc:oc + 1],
                            woblk(oc, h),
                            attn[:, h:h + 1],
                            start=(h == 0), stop=(h == REPEATS - 1),
                        )
                # split drain: bulk oc groups pre-run while the wo ladders'
                # mid/last blocks stream; only a 9-col drain sits in the tail
                nc.vector.tensor_copy(o_sb[:, 0:23], pout[:, 0:23])
                nc.vector.tensor_copy(o_sb[:, 23:], pout[:, 23:])
                chain(0, nc.sync.dma_start(out=out_p[:], in_=o_sb[:]))

    nc.compile()
    # Trim the program epilogue to just SP's completion waits (all DMA-queue
    # and engine sems, which include the output DMA) plus its drain; drop
    # both all-engine barrier rounds and the sem-reset ISA (~1us of pure
    # sem cascade).  Single-shot execution doesn't need the reset.
    end = nc.m.functions[0].blocks[-1]
    keep = []
    for inst in end.instructions:
        if inst.engine != mybir.EngineType.SP or isinstance(inst, mybir.InstDrain):
            continue
        si = inst.sync_info
        if si is None or not any(
                (w.ant_name or "").startswith("DMAHW0") for w in si.on_wait):
            continue
        keep.append(inst)
    assert keep, "expected an SP wait on its HWDGE queue sem"
    end.instructions = keep
    return nc


def _shard_inputs(x, wq, wk, wv, wo, cache_k, cache_v, cos, sin):
    """Build the 8 per-core input maps (fp16 weights/KV, C-contiguous)."""
    wq_off, kt_off, v_off, mega_cols, p1_cols = _mega_layout()

    x_flat = np.asarray(x, dtype=np.float32).reshape(DIM)
    x_col = x_flat.reshape(KCH, 128).T.astype(np.float16)  # [128, 32]

    cos = np.asarray(cos, np.float32).reshape(-1)  # [64]
    sin = np.asarray(sin, np.float32).reshape(-1)
    # rot = R.T (matmul lhsT layout) for the block-diag 2x2 rotation R
    rot = np.zeros((128, 128), np.float32)
    i = np.arange(64)
    rot[2 * i, 2 * i] = cos
    rot[2 * i + 1, 2 * i + 1] = cos
    rot[2 * i + 1, 2 * i] = -sin
    rot[2 * i, 2 * i + 1] = sin
    xtra = np.concatenate(
        [x_col, rot.astype(np.float16), np.eye(128, dtype=np.float16)], axis=1)

    wq = np.asarray(wq, np.float32)
    wk = np.asarray(wk, np.float32)
    wv = np.asarray(wv, np.float32)
    wo = np.asarray(wo, np.float32)
    cache_k = np.asarray(cache_k, np.float32)
    cache_v = np.asarray(cache_v, np.float32)

    in_maps = []
    for c in range(N_CORES):
        wq_c = wq[c * QCOLS:(c + 1) * QCOLS]              # [512, 4096]
        wk_c = wk[c * HEAD_DIM:(c + 1) * HEAD_DIM]        # [128, 4096]
        wv_c = wv[c * HEAD_DIM:(c + 1) * HEAD_DIM]
        q_blk = (wq_c.reshape(REPEATS, 128, KCH, 128)
                 .transpose(2, 3, 0, 1).reshape(KCH, 128, QCOLS))
        k_blk = wk_c.reshape(128, KCH, 128).transpose(1, 2, 0)
        v_blk = wv_c.reshape(128, KCH, 128).transpose(1, 2, 0)
        wqkv_c = np.concatenate([q_blk, k_blk, v_blk], axis=2)  # [32,128,768]
        wqkv_c = wqkv_c.astype(np.float16)
        # chunk 63 slot rotation: slot 0 <- new position (device-written),
        # slots 1..127 <- cache positions 8064..8190
        kraw = cache_k[0, :KV_LEN, c, :].T  # [128, 8192]
        k_c = np.empty((128, KV_LEN), np.float16)
        k_c[:, :KV_LEN - 128] = kraw[:, :KV_LEN - 128]
        k_c[:, KV_LEN - 128] = 0
        k_c[:, KV_LEN - 127:] = kraw[:, KV_LEN - 128:KV_LEN - 1]
        vraw = cache_v[0, :KV_LEN, c, :]  # [8192, 128]
        v_c = np.empty((TCH, 128, HEAD_DIM), np.float16)
        v_c[:TCH - 1] = vraw[:KV_LEN - 128].reshape(TCH - 1, 128, HEAD_DIM)
        v_c[TCH - 1, 0] = 0
        v_c[TCH - 1, 1:] = vraw[KV_LEN - 128:KV_LEN - 1]
        v_c = v_c.transpose(1, 0, 2)  # [128, 64, 128]

        m = {}
        for q in range(3):
            parts = []
            if q == 2:
                parts.append(xtra)
            for cc in range(*W_SPLIT[q]):
                parts.append(wqkv_c[cc])
            lo, hi = KT_SPLIT[q]
            parts.append(k_c[:, lo * 128:hi * 128])
            lo, hi = V_SPLIT[q]
            parts.append(v_c[:, lo:hi].reshape(128, (hi - lo) * 128))
            m[f"s{q}"] = np.ascontiguousarray(np.concatenate(parts, axis=1))
            assert m[f"s{q}"].shape[1] == mega_cols[q]
        wo_c = wo[:, c * QCOLS:(c + 1) * QCOLS].astype(np.float16)  # [4096,512]
        for q, bl in enumerate(WO_BLOCKS):
            blocks = []
            for b in bl:
                oc, h = b // REPEATS, b % REPEATS
                blocks.append(
                    wo_c[oc * 128:(oc + 1) * 128, h * 128:(h + 1) * 128].T)
            m[f"wo{q}"] = np.ascontiguousarray(np.concatenate(blocks, axis=1))
        in_maps.append(m)
    return in_maps


def get_program(reps=1):
    if "nc" not in _CACHED:
        _CACHED["nc"] = _build()
    return _CACHED["nc"]


def kernel(x, wq, wk, wv, wo, cache_k, cache_v, cos, sin, start_pos):
    nc = get_program()
    in_maps = _shard_inputs(x, wq, wk, wv, wo, cache_k, cache_v, cos, sin)
    res = run_bass_kernel_spmd(nc, in_maps, list(range(N_CORES)))
    out = np.zeros(DIM, np.float32)
    for c in range(N_CORES):
        out += res.results[c]["out_p"].T.reshape(DIM)
    return out.reshape(1, 1, DIM)
